# revision 11
# baseline (speedup 1.0000x reference)
"""Trainium2 Bass kernel for a 2-layer LSTM encoder.

Model: out = LSTM2(LSTM1(emb[x])) with B=64, S=512, E=256, U=1024, V=32000.

Strategy (8 NeuronCores):
  * 8-way tensor (unit) parallelism, batch replicated. Core r owns units
    [128r, 128r+128) of BOTH layers, i.e. 512 gate columns per layer
    (gate order reordered host-side to [i, f, o, g]).
  * The embedding lookup is fused with the layer-1 input projection:
    device computes E1 = emb @ Wk1_shard + b1_shard ([32000, 512] bf16)
    once, then per time step an indirect DMA gathers the 64 rows E1[x[:,t]]
    (this IS x_t @ Wk1 + b1 for the shard).
  * Fused per-step loop: one step computes layer1(t) and layer2(t-1),
    then transposes the two fresh h-shards ([64,128] -> [128,64]) on the
    PE and AllGathers both in a single collective so every core has the
    full h1(t) / h2(t-1) (as stacked K-chunk lhsT tiles) for the next step.
  * Matmuls run batch-as-M (M=64) with 2-way PE column tiling: array
    columns 0-63 compute gate cols [i,f], columns 64-127 compute [o,g]
    concurrently -> full 128x128 array utilization at bf16.
  * Elementwise: sigmoid(i,o) is a single [128,128] ACT op (i on psum rows
    0-63, o on rows 64-127); f/g half-ops; c stays fp32.

kernel(**inputs) takes the FULL numpy inputs and returns the FULL
[64, 512, 1024] float32 output.
"""

import os
import sys

for _p in ("/opt/trn_rl_repo", "/root/.axon_site/_ro/trn_rl_repo"):
    if os.path.isdir(_p) and _p not in sys.path:
        sys.path.insert(0, _p)

import numpy as np
import ml_dtypes

BF16 = ml_dtypes.bfloat16

B = 64
S_FULL = 512
E = 256
U = 1024
V = 32000
NC = 8
USH = U // NC          # 128 units per core per layer
GC = 4 * USH           # 512 gate columns per core
KCH = U // 128         # 8 K-chunks over the hidden dim


def _build_program(S):
    import concourse.bass as bass
    import concourse.bacc as bacc
    import concourse.mybir as mybir
    import concourse.tile as tile

    f32 = mybir.dt.float32
    bf16 = mybir.dt.bfloat16
    i32 = mybir.dt.int32
    AF = mybir.ActivationFunctionType

    nc = bacc.Bacc("TRN2", target_bir_lowering=False, debug=False, num_devices=NC)

    # ---- kernel I/O ----
    embT = nc.dram_tensor("embT", [E, V], bf16, kind="ExternalInput")
    wk1s = nc.dram_tensor("wk1s", [E, GC], bf16, kind="ExternalInput")
    wr1s = nc.dram_tensor("wr1s", [U, GC], bf16, kind="ExternalInput")
    wk2s = nc.dram_tensor("wk2s", [U, GC], bf16, kind="ExternalInput")
    wr2s = nc.dram_tensor("wr2s", [U, GC], bf16, kind="ExternalInput")
    b1t_d = nc.dram_tensor("b1t", [128, GC], f32, kind="ExternalInput")
    b2t_d = nc.dram_tensor("b2t", [B, GC], bf16, kind="ExternalInput")
    idnb_d = nc.dram_tensor("idnb", [B, B], bf16, kind="ExternalInput")
    idnf_d = nc.dram_tensor("idnf", [B, B], f32, kind="ExternalInput")
    xidx_d = nc.dram_tensor("xidx", [B, S], i32, kind="ExternalInput")
    outp = nc.dram_tensor("outp", [S, B, USH], f32, kind="ExternalOutput")

    # internal DRAM
    E1 = nc.dram_tensor("E1", [V, GC], bf16)
    # Shared-address-space AllGather outputs (2 parities per h tag);
    # shared outputs make the 8-core AllGather ~3x faster than Local.
    agout_sh = {
        tag: [
            nc.dram_tensor(f"agout_{tag}_{p}", [NC, 128, B], bf16, addr_space="Shared")
            for p in range(2)
        ]
        for tag in ("1", "2")
    }

    with tile.TileContext(nc) as tc:
        with (
            tc.tile_pool(name="const", bufs=1) as cp,
            tc.tile_pool(name="dram", bufs=3, space="DRAM") as dp,
        ):
            # resident weights: [128, KCH, GC] with chunk k at [:, k, :]
            wr1_sb = cp.tile([128, KCH, GC], bf16)
            wk2_sb = cp.tile([128, KCH, GC], bf16)
            wr2_sb = cp.tile([128, KCH, GC], bf16)
            wk1_sb = cp.tile([128, 2, GC], bf16)
            nc.sync.dma_start(wr1_sb[:], wr1s[:].rearrange("(k p) n -> p k n", p=128))
            nc.sync.dma_start(wk2_sb[:], wk2s[:].rearrange("(k p) n -> p k n", p=128))
            nc.sync.dma_start(wr2_sb[:], wr2s[:].rearrange("(k p) n -> p k n", p=128))
            nc.sync.dma_start(wk1_sb[:], wk1s[:].rearrange("(k p) n -> p k n", p=128))
            b1t = cp.tile([128, GC], f32)
            b2t = cp.tile([B, GC], bf16)
            idnb = cp.tile([B, B], bf16)
            idnf = cp.tile([B, B], f32)
            xidx = cp.tile([B, S], i32)
            nc.sync.dma_start(b1t[:], b1t_d[:])
            nc.sync.dma_start(b2t[:], b2t_d[:])
            nc.sync.dma_start(idnb[:], idnb_d[:])
            nc.sync.dma_start(idnf[:], idnf_d[:])
            nc.sync.dma_start(xidx[:], xidx_d[:])

            # ---- phase 1: E1 = emb @ Wk1_shard + b1 ----
            with (
                tc.tile_pool(name="embp", bufs=3) as ep,
                tc.tile_pool(name="emps", bufs=2, space="PSUM") as eps,
                nc.named_scope("e1phase"),
            ):
                # 4 token-tiles per embT load: [128, 512] super-tiles give the
                # DMA 1KB/partition contiguous instead of 256B
                for sm in range((V + 511) // 512):
                    base = sm * 512
                    nt = min(4, (V - base) // 128)  # token-tiles in this block
                    l0 = ep.tile([128, 4, 128], bf16, tag="l0")
                    l1 = ep.tile([128, 4, 128], bf16, tag="l1")
                    nc.sync.dma_start(
                        l0[:, 0:nt, :].rearrange("p k n -> p (k n)"),
                        embT[0:128, base:base + nt * 128])
                    nc.sync.dma_start(
                        l1[:, 0:nt, :].rearrange("p k n -> p (k n)"),
                        embT[128:256, base:base + nt * 128])
                    for j in range(nt):
                        m = sm * 4 + j
                        ps = eps.tile([128, GC], f32)
                        nc.tensor.matmul(ps[:], l0[:, j, :], wk1_sb[:, 0, :],
                                         start=True, stop=False)
                        nc.tensor.matmul(ps[:], l1[:, j, :], wk1_sb[:, 1, :],
                                         start=False, stop=True)
                        e1t = ep.tile([128, GC], bf16, tag="e1t")
                        nc.vector.tensor_add(e1t[:], ps[:], b1t[:])
                        nc.sync.dma_start(E1[m * 128:(m + 1) * 128, :], e1t[:])

            # ---- phase 2: fused recurrent loop ----
            with (
                tc.tile_pool(name="gath", bufs=8) as gp,
                tc.tile_pool(name="ew", bufs=4) as wp,
                tc.tile_pool(name="st", bufs=4) as sp,
                tc.tile_pool(name="psz", bufs=2, space="PSUM") as pz,
                tc.tile_pool(name="pst", bufs=2, space="PSUM") as pt,
                nc.named_scope("steploop"),
            ):
                PF = 5  # gather prefetch depth

                def gather(t):
                    g = gp.tile([B, GC], bf16, tag="g1")
                    nc.gpsimd.indirect_dma_start(
                        out=g[:],
                        out_offset=None,
                        in_=E1[:],
                        in_offset=bass.IndirectOffsetOnAxis(ap=xidx[:, t:t + 1], axis=0),
                    )
                    return g

                # initial state
                h1Ta0 = sp.tile([128, KCH // 2, B], bf16, tag="h1Ta")
                h1Tb0 = sp.tile([128, KCH // 2, B], bf16, tag="h1Tb")
                h2Ta0 = sp.tile([128, KCH // 2, B], bf16, tag="h2Ta")
                h2Tb0 = sp.tile([128, KCH // 2, B], bf16, tag="h2Tb")
                h1T = (h1Ta0, h1Tb0)
                h2T = (h2Ta0, h2Tb0)
                c1 = sp.tile([B, USH], f32, tag="c1")
                c2 = sp.tile([B, USH], f32, tag="c2")
                for _t in (*h1T, *h2T):
                    nc.gpsimd.memset(_t[:], 0.0)
                nc.gpsimd.memset(c1[:], 0.0)
                nc.gpsimd.memset(c2[:], 0.0)

                gq = [gather(t) for t in range(min(PF, S))]

                H = GC // 4  # 128, gate width

                def ew(z, c_cur, h_dtype, out_pool_tag):
                    """z: psum [128, 2H]; rows 0-63 = [i,f], rows 64-127 = [o,g].
                    Returns (h [B,USH] h_dtype, c_new [B,USH] f32).
                    ACT order: sif first (starts the f*c mul earliest), then
                    g, then o; tanh(c) last."""
                    sif = wp.tile([B, 2 * H], f32, tag="sif" + out_pool_tag)
                    nc.scalar.activation(sif[:], z[0:B, :], AF.Sigmoid)
                    t1 = wp.tile([B, USH], f32, tag="t1" + out_pool_tag)
                    nc.vector.tensor_mul(t1[:], sif[:, H:2 * H], c_cur[:])
                    gt = wp.tile([B, H], f32, tag="gt" + out_pool_tag)
                    nc.scalar.activation(gt[:], z[B:128, H:2 * H], AF.Tanh)
                    t2 = wp.tile([B, USH], f32, tag="t2" + out_pool_tag)
                    nc.vector.tensor_mul(t2[:], sif[:, 0:H], gt[:])
                    so = wp.tile([B, H], f32, tag="so" + out_pool_tag)
                    nc.scalar.activation(so[:], z[B:128, 0:H], AF.Sigmoid)
                    c_new = sp.tile([B, USH], f32, tag="c" + out_pool_tag)
                    nc.vector.tensor_add(c_new[:], t1[:], t2[:])
                    tcn = wp.tile([B, USH], f32, tag="tc" + out_pool_tag)
                    nc.scalar.activation(tcn[:], c_new[:], AF.Tanh)
                    h = wp.tile([B, USH], h_dtype, tag="h" + out_pool_tag)
                    nc.vector.tensor_mul(h[:], so[:], tcn[:])
                    return h, c_new

                def zmm(z, pairs):
                    """Accumulate into z psum [128, 2H] with 2-way col tiling.
                    pairs: list of (lhsT_ap, rhs_full_ap) where rhs covers GC
                    cols; left half -> rows 0:64, right half -> rows 64:128."""
                    n = len(pairs)
                    for idx, (lh, rh) in enumerate(pairs):
                        st = idx == 0
                        sp_ = idx == n - 1
                        nc.tensor.matmul(
                            z[0:B, :], lh, rh[:, 0:2 * H], start=st, stop=sp_,
                            tile_position=(0, 0), skip_group_check=True)
                        nc.tensor.matmul(
                            z[B:128, :], lh, rh[:, 2 * H:4 * H], start=st, stop=sp_,
                            tile_position=(0, B), skip_group_check=True)

                def bcast(hloc, tag, par):
                    """Bounce hloc [128,B] to DRAM, AllGather into a Shared
                    output, unpack into TWO half-tiles (chunks 0-3 / 4-7) so
                    the first chunks' matmuls can start while the second half
                    is still landing."""
                    agin = dp.tile([128, B], bf16, tag="agin" + tag)
                    nc.sync.dma_start(agin[:], hloc[:])
                    agout = agout_sh[tag][par]
                    nc.gpsimd.collective_compute(
                        "AllGather",
                        mybir.AluOpType.bypass,
                        replica_groups=[list(range(NC))],
                        ins=[agin[:]],
                        outs=[agout[:]],
                    )
                    hTa = sp.tile([128, KCH // 2, B], bf16, tag="h" + tag + "Ta")
                    hTb = sp.tile([128, KCH // 2, B], bf16, tag="h" + tag + "Tb")
                    nc.sync.dma_start(
                        hTa[:], agout[0:KCH // 2, :, :].rearrange("k p c -> p k c"))
                    nc.sync.dma_start(
                        hTb[:], agout[KCH // 2:KCH, :, :].rearrange("k p c -> p k c"))
                    return (hTa, hTb)

                def hch(hT, k):
                    a, b = hT
                    return a[:, k, :] if k < KCH // 2 else b[:, k - KCH // 2, :]

                for t in range(S + 1):
                    h1T_old, h2T_old = h1T, h2T

                    if t < S:
                        # ---- layer 1, step t ----
                        g1 = gq.pop(0)
                        z1 = pz.tile([128, 2 * H], f32, tag="z1")
                        zmm(z1, [(idnb[:], g1[:])] +
                               [(hch(h1T_old, k), wr1_sb[:, k, :]) for k in range(KCH)])
                        h1b, c1 = ew(z1, c1, bf16, "1")
                        # transpose + all-gather h1(t) as early as possible
                        p1 = pt.tile([128, B], bf16, tag="p1")
                        nc.tensor.transpose(p1[:], h1b[:], idnb[:])
                        h1L = wp.tile([128, B], bf16, tag="h1L")
                        nc.vector.tensor_copy(h1L[:], p1[:])
                        h1T = bcast(h1L, "1", t % 2)

                    if t >= 1:
                        # ---- layer 2, step t-1 ----
                        z2 = pz.tile([128, 2 * H], f32, tag="z2")
                        zmm(z2, [(idnb[:], b2t[:])] +
                               [(hch(h1T_old, k), wk2_sb[:, k, :]) for k in range(KCH)] +
                               [(hch(h2T_old, k), wr2_sb[:, k, :]) for k in range(KCH)])
                        h2f, c2 = ew(z2, c2, f32, "2")
                        nc.sync.dma_start(outp[t - 1, :, :], h2f[:])
                        if t < S:
                            p2 = pt.tile([128, B], f32, tag="p2")
                            nc.tensor.transpose(p2[:], h2f[:], idnf[:])
                            h2L = wp.tile([128, B], bf16, tag="h2L")
                            nc.vector.tensor_copy(h2L[:], p2[:])
                            h2T = bcast(h2L, "2", t % 2)

                    # prefetch gather AFTER the collective doorbells so the
                    # 1us indirect-DMA descgen doesn't delay the AG triggers
                    # in the gpsimd queue
                    if t + PF < S:
                        gq.append(gather(t + PF))

    nc.compile()
    return nc


_CACHE = {}


def _get_program(S):
    if S not in _CACHE:
        _CACHE[S] = _build_program(S)
    return _CACHE[S]


def _gate_perm_cols(r):
    """Column indices of core r's shard in keras gate order [i,f,g,o] ->
    device order [i,f,o,g]."""
    sl = np.arange(r * USH, (r + 1) * USH)
    return np.concatenate([0 * U + sl, 1 * U + sl, 3 * U + sl, 2 * U + sl])


def make_in_maps(x, emb, Wk1, Wr1, b1, Wk2, Wr2, b2, S):
    embT = np.ascontiguousarray(emb.T).astype(BF16)
    idnb = np.eye(B, dtype=BF16)
    idnf = np.eye(B, dtype=np.float32)
    xidx = np.ascontiguousarray(x[:, :S]).astype(np.int32)
    in_maps = []
    for r in range(NC):
        cols = _gate_perm_cols(r)
        b1s = b1[cols].astype(np.float32)
        b2s = b2[cols].astype(BF16)
        in_maps.append({
            "embT": embT,
            "wk1s": np.ascontiguousarray(Wk1[:, cols]).astype(BF16),
            "wr1s": np.ascontiguousarray(Wr1[:, cols]).astype(BF16),
            "wk2s": np.ascontiguousarray(Wk2[:, cols]).astype(BF16),
            "wr2s": np.ascontiguousarray(Wr2[:, cols]).astype(BF16),
            "b1t": np.ascontiguousarray(np.broadcast_to(b1s, (128, GC))),
            "b2t": np.ascontiguousarray(np.broadcast_to(b2s, (B, GC))),
            "idnb": idnb,
            "idnf": idnf,
            "xidx": xidx,
        })
    return in_maps


def assemble_output(results, S):
    out = np.empty((B, S, U), dtype=np.float32)
    for r in range(NC):
        out[:, :, r * USH:(r + 1) * USH] = results[r]["outp"].transpose(1, 0, 2)
    return out


def run(inputs, trace=False, S=S_FULL, trace_kwargs=None):
    from concourse import bass_utils
    nc = _get_program(S)
    in_maps = make_in_maps(
        np.asarray(inputs["x"]), np.asarray(inputs["emb"]),
        np.asarray(inputs["Wk1"]), np.asarray(inputs["Wr1"]), np.asarray(inputs["b1"]),
        np.asarray(inputs["Wk2"]), np.asarray(inputs["Wr2"]), np.asarray(inputs["b2"]),
        S)
    res = bass_utils.run_bass_kernel_spmd(
        nc, in_maps, core_ids=list(range(NC)), trace=trace,
        **(trace_kwargs or {}))
    return assemble_output(res.results, S), res


def kernel(x, emb, Wk1, Wr1, b1, Wk2, Wr2, b2):
    out, _ = run(dict(x=x, emb=emb, Wk1=Wk1, Wr1=Wr1, b1=b1,
                      Wk2=Wk2, Wr2=Wr2, b2=b2))
    return out

